# revision 36
# baseline (speedup 1.0000x reference)
"""Trainium2 Bass kernel for nn_Attention (dense transformer attention w/ gating).

Sharding (8 cores, hardcoded): 2 q-row blocks (512 rows) x 4 batch groups (2
batches). Each core computes full attention for its (q-rows, batches) slice for
all 8 heads. No collectives; host shards inputs / gathers outputs.

Layout: softmax axis (k) is the partition dim so the P@V matmul needs no
transposes. probs = exp(qk) * exp(bias + nonbatched_bias) with the bias factor
(ebn) precomputed on host (multiplicative softmax factoring). Denominator via a
"2.0 column" appended to V (row 32 of each PV psum block); 1/(2d) = 0.5/d folds
the 0.5 of sigmoid(x) = 0.5*tanh(x/2) + 0.5, so the gate multiply is a single
(tanh + 1) * pv scalar_tensor_tensor with zero rows under the denominator.

Schedule: the ACT engine (exp) is the bottleneck (~79us of activations), so the
whole kernel is built around keeping its FIFO gapless: a dummy activation
preloads the exp table during the DMA ramp; only the 6 matmuls feeding the
first head-pair precede attention; all other projections drip into the pair
loop one small thunk per QK chunk; PV runs one pair behind QK/exp so the PE
always has ready work; all input DMAs issue from the (otherwise idle) sync
engine with the first-pair inputs leading the ring, outputs via SWDGE.
"""
import numpy as np
import ml_dtypes

import concourse.bass as bass
import concourse.mybir as mybir
import concourse.tile as tile

B, NQ, NK, D, H = 8, 1024, 1024, 256, 8
DK = DV = 32
GI, GJ = 2, 4          # q-row blocks x batch groups
RQ = NQ // GI          # 512 q rows per core
BC = B // GJ           # 2 batches per core
KC = NK // 128         # 8 k chunks
N_CORES = 8
VW = 34                # per-head column stride in the augmented V tile
FKR = KC * RQ          # 4096 probs columns per head

bf16 = mybir.dt.bfloat16
f32 = mybir.dt.float32
AF = mybir.ActivationFunctionType
OP = mybir.AluOpType


def _split_waits(nc, limit=1):
    """walrus here only allows 1 sync-wait per instruction: hoist extras
    onto same-engine NoOps inserted just before."""
    for f in nc.m.functions:
        for bb in f.blocks:
            new_insts = []
            for inst in bb.instructions:
                si = inst.sync_info
                if si and si.on_wait and len(si.on_wait) > limit:
                    extra = si.on_wait[limit:]
                    si.on_wait = si.on_wait[:limit]
                    for i, w in enumerate(extra):
                        new_insts.append(mybir.InstNoOp(
                            name=f"{inst.name}-ws{i}", ins=[], outs=[],
                            engine=inst.engine,
                            sync_info=mybir.SyncInfo(on_wait=[w], on_update=[]),
                        ))
                new_insts.append(inst)
            bb.instructions[:] = new_insts


def _build_nc():
    nc = bass.Bass()
    # acts: per (batch, a-chunk) mt [128,1024] || qt [128,512]
    acts_d = nc.dram_tensor("acts", [BC, 2, 128, NK + RQ], bf16,
                            kind="ExternalInput")
    # pair-major: ebn[(b*4+p)][part, h*FKR + kc*RQ + q] for the pair's heads
    ebn_d = nc.dram_tensor("ebn", [BC * H // 2, 128, 2 * FKR], bf16,
                           kind="ExternalInput")
    wkq_d = nc.dram_tensor("wkq", [2, 128, 2 * D], bf16, kind="ExternalInput")
    wvg_d = nc.dram_tensor("wvg", [2, 128, 2 * D], bf16, kind="ExternalInput")
    gb_d = nc.dram_tensor("gb", [2, 128, 1], f32, kind="ExternalInput")
    # per (batch, head-pair): rows 0-31 wavg_even, 32 den_even, 64-95 wavg_odd,
    # 96 den_odd -> shipped as [2 duo, 33, RQ]
    outw_d = nc.dram_tensor("outw", [BC, H // 2, 2, 33, RQ], f32,
                            kind="ExternalOutput")

    with tile.TileContext(nc) as tc:
        with (
            tc.tile_pool(name="weights", bufs=1) as wpool,
            tc.tile_pool(name="acts", bufs=2) as apool,
            tc.tile_pool(name="ebn", bufs=3) as epool,
            tc.tile_pool(name="eqk", bufs=2) as qpool,
            tc.tile_pool(name="probs", bufs=2) as prpool,
            tc.tile_pool(name="outs", bufs=2) as opool,
            tc.tile_pool(name="pj", bufs=2, space="PSUM") as pj_pool,
            tc.tile_pool(name="pl", bufs=3, space="PSUM") as pl_pool,
        ):
            # --- ACT exp-table preload: a dummy activation first in the
            # scalar FIFO so the ~2.7us table load overlaps the DMA ramp ---
            warm = wpool.tile([128, 1], f32, name="warm", tag="warm")
            nc.gpsimd.memset(warm[:], 0.0)
            nc.scalar.activation(warm[:], warm[:], AF.Exp)

            # --- resident weights; ALL input DMAs issue from the sync engine
            # (scalar stays pure-ACT): wkq + b0 acts + first ebn lead the ring
            wkq_sb = [wpool.tile([128, 2 * D], bf16, name=f"wkq{a}", tag=f"wkq{a}") for a in range(2)]
            wvg_sb = [wpool.tile([128, 2 * D], bf16, name=f"wvg{a}", tag=f"wvg{a}") for a in range(2)]
            gb_sb = [wpool.tile([128, 1], f32, name=f"gb{g}", tag=f"gb{g}") for g in range(2)]
            for a in range(2):
                nc.sync.dma_start(out=wkq_sb[a][:], in_=wkq_d[a])

            qt_all, mt_all = [None] * BC, [None] * BC
            kt_all, va_all, qh_all, g01_all = [None] * BC, [None] * BC, [None] * BC, [None] * BC

            AW = NK + RQ
            def emit_load(b):
                am = apool.tile([128, 2 * AW], bf16, name="acts", tag="acts")
                if b == 0:
                    # critical path: land mt(a0) first so K-proj starts sooner
                    nc.sync.dma_start(out=am[:, 0:NK], in_=acts_d[b, 0, :, 0:NK])
                    nc.sync.dma_start(out=am[:, NK:AW], in_=acts_d[b, 0, :, NK:AW])
                    nc.sync.dma_start(out=am[:, AW:2 * AW], in_=acts_d[b, 1])
                else:
                    for a in range(2):
                        nc.sync.dma_start(out=am[:, a * AW:(a + 1) * AW],
                                          in_=acts_d[b, a])
                qt_all[b], mt_all[b] = am, am

            def emit_kproj(b, gs=(0, 1)):
                mt_sb = mt_all[b]
                if kt_all[b] is None:
                    kt_all[b] = [apool.tile([128, NK], bf16,
                                            name=f"kt{g}", tag=f"kt{g}")
                                 for g in range(2)]
                kt_sb = kt_all[b]
                for g in gs:
                    for n2 in range(2):
                        ps = pj_pool.tile([128, 512], f32, name="pj", tag="pj")
                        for a in range(2):
                            nc.tensor.matmul(
                                out=ps[:],
                                lhsT=wkq_sb[a][:, g * 128:(g + 1) * 128],
                                rhs=mt_sb[:, a * AW + n2 * 512: a * AW + (n2 + 1) * 512],
                                start=(a == 0), stop=(a == 1))
                        nc.vector.tensor_copy(
                            kt_sb[g][:, n2 * 512:(n2 + 1) * 512], ps[:])

            def emit_qproj(b, gs=(0, 1)):
                qt_sb = qt_all[b]
                if qh_all[b] is None:
                    qh_all[b] = [apool.tile([128, RQ], bf16,
                                            name=f"qh{g}", tag=f"qh{g}")
                                 for g in range(2)]
                qh_sb = qh_all[b]
                for g in gs:
                    ps = pj_pool.tile([128, 512], f32, name="pj", tag="pj")
                    for a in range(2):
                        nc.tensor.matmul(
                            out=ps[:],
                            lhsT=wkq_sb[a][:, D + g * 128:D + (g + 1) * 128],
                            rhs=qt_sb[:, a * AW + NK:(a + 1) * AW],
                            start=(a == 0), stop=(a == 1))
                    nc.vector.tensor_copy(qh_sb[g][:], ps[:])

            def emit_vproj(b, kcs):
                mt_sb = mt_all[b]
                if va_all[b] is None:
                    va_all[b] = [apool.tile([128, 8 * VW], bf16,
                                            name=f"va{kc}", tag=f"va{kc}")
                                 for kc in range(KC)]
                va_sb = va_all[b]
                for kc in kcs:
                    nc.gpsimd.memset(va_sb[kc][:, 32:8 * VW:VW], 2.0)
                    ps = pj_pool.tile([128, 512], f32, name="pj", tag="pj")
                    for a in range(2):
                        nc.tensor.matmul(
                            out=ps[:, 0:D],
                            lhsT=mt_sb[:, a * AW + kc * 128: a * AW + (kc + 1) * 128],
                            rhs=wvg_sb[a][:, 0:D],
                            start=(a == 0), stop=(a == 1))
                    dst = va_sb[kc][:].rearrange("p (h x) -> p h x", h=8)[:, :, 0:32]
                    src = ps[:, 0:D].rearrange("p (h x) -> p h x", h=8)
                    nc.vector.tensor_copy(dst, src)

            def emit_gate(b, gs=(0, 1)):
                qt_sb = qt_all[b]
                if g01_all[b] is None:
                    g01_all[b] = [apool.tile([128, RQ], bf16,
                                             name=f"g01{g}", tag=f"g01{g}")
                                  for g in range(2)]
                g01_sb = g01_all[b]
                for g in gs:
                    ps2 = pj_pool.tile([128, 512], f32, name="pj", tag="pj")
                    for a in range(2):
                        nc.tensor.matmul(
                            out=ps2[:],
                            lhsT=wvg_sb[a][:, D + g * 128:D + (g + 1) * 128],
                            rhs=qt_sb[:, a * AW + NK:(a + 1) * AW],
                            start=(a == 0), stop=(a == 1))
                    # sigmoid = (tanh(0.5*x + 0.5*gb) + 1) * 0.5; the *0.5 is
                    # folded into the 2.0 denominator column
                    nc.scalar.activation(g01_sb[g][:], ps2[:], AF.Tanh,
                                         bias=gb_sb[g][:], scale=0.5)

            def load_ebn(b, p):
                ebn_sb = epool.tile([128, 2 * FKR], bf16, name="ebn", tag="ebn")
                nc.sync.dma_start(out=ebn_sb[:], in_=ebn_d[b * 4 + p])
                return ebn_sb

            def pair_front(b, p, ebn_pre=None, extras=()):
                extras = list(extras)
                kt_sb, qh_sb = kt_all[b], qh_all[b]
                g = p // 2
                # 2MB of ebn for the pair in one transfer
                ebn_sb = ebn_pre if ebn_pre is not None else load_ebn(b, p)

                eqk = [qpool.tile([128, FKR], bf16, name=f"eqk{i}", tag=f"eqk{i}")
                       for i in range(2)]
                probs = [prpool.tile([128, FKR], bf16, name=f"probs{i}",
                                     tag=f"probs{i}") for i in range(2)]
                # two k chunks per psum logit tile; the two heads' matmuls
                # interleave so their 32-row strips overlap on the PE array
                for c0, c1 in ((0, 2), (2, 4), (4, 6), (6, 8)):
                    w = (c1 - c0) * RQ
                    sl = slice(c0 * RQ, c1 * RQ)
                    pls = [pl_pool.tile([128, 2 * RQ], f32, name="pl", tag="pl")
                           for _ in range(2)]
                    for kc2 in range(c1 - c0):
                        kc = c0 + kc2
                        for idx in range(2):
                            strip = (2 * p + idx) % 4
                            nc.tensor.matmul(
                                out=pls[idx][:, kc2 * RQ:(kc2 + 1) * RQ],
                                lhsT=kt_sb[g][32 * strip:32 * (strip + 1),
                                              kc * 128:(kc + 1) * 128],
                                rhs=qh_sb[g][32 * strip:32 * (strip + 1), :],
                                start=True, stop=True,
                                tile_position=(32 * strip, 0))
                    for idx in range(2):
                        nc.scalar.activation(
                            eqk[idx][:, sl], pls[idx][:, 0:w], AF.Exp)
                    if c1 in (4, 8):
                        hsl = slice((c1 - 4) * RQ, c1 * RQ)
                        for idx in range(2):
                            nc.vector.tensor_tensor(
                                out=probs[idx][:, hsl], in0=eqk[idx][:, hsl],
                                in1=ebn_sb[:, idx * FKR + (c1 - 4) * RQ:
                                           idx * FKR + c1 * RQ],
                                op=OP.mult)
                    if extras:
                        extras.pop(0)()
                while extras:
                    extras.pop(0)()
                return (b, p, probs)

            def emit_pv(st):
                b, p, probs = st
                va_sb = va_all[b]
                g01_sb = g01_all[b]
                g, s0 = p // 2, 2 * (p % 2)
                # pair gate tile: rows 0-31 tanh(even head), 64-95 tanh(odd),
                # rows 32/96 zero so the den rows pass through (tanh+1=1)
                g01x = apool.tile([128, RQ], bf16, name="g01x", tag="g01x")
                nc.gpsimd.memset(g01x[32:33, :], 0.0)
                nc.gpsimd.memset(g01x[96:97, :], 0.0)
                nc.vector.tensor_copy(
                    g01x[0:32, :], g01_sb[g][32 * s0:32 * s0 + 32, :])
                nc.vector.tensor_copy(
                    g01x[64:96, :], g01_sb[g][32 * s0 + 32:32 * s0 + 64, :])
                ppv = pj_pool.tile([128, RQ], f32, name="pv", tag="pj")
                he, ho = 2 * p, 2 * p + 1
                for kc in range(KC):
                    nc.tensor.matmul(
                        out=ppv[0:33, :],
                        lhsT=va_sb[kc][:, he * VW:he * VW + 33],
                        rhs=probs[0][:, kc * RQ:(kc + 1) * RQ],
                        start=(kc == 0), stop=(kc == KC - 1),
                        tile_position=(0, 0))
                    nc.tensor.matmul(
                        out=ppv[64:97, :],
                        lhsT=va_sb[kc][:, ho * VW:ho * VW + 33],
                        rhs=probs[1][:, kc * RQ:(kc + 1) * RQ],
                        start=(kc == 0), stop=(kc == KC - 1),
                        tile_position=(0, 64))
                # gated wavg + untouched den rows in one pass
                wavgx = opool.tile([128, RQ], f32, name="wavgx", tag="wavgx")
                nc.vector.scalar_tensor_tensor(
                    out=wavgx[0:97, :], in0=g01x[0:97, :], scalar=1.0,
                    in1=ppv[0:97, :], op0=OP.add, op1=OP.mult)
                nc.gpsimd.dma_start(out=outw_d[b, p, 0], in_=wavgx[0:33, :])
                nc.gpsimd.dma_start(out=outw_d[b, p, 1], in_=wavgx[64:97, :])

            # --- schedule: minimal critical path to the first exp; all other
            # projection work drips into the pair fronts one small thunk per
            # QK chunk, so the PE FIFO never carries a long non-QK block ---
            emit_load(0)
            eb00 = load_ebn(0, 0)
            for a in range(2):
                nc.sync.dma_start(out=wvg_sb[a][:], in_=wvg_d[a])
                nc.sync.dma_start(out=gb_sb[a][:], in_=gb_d[a])
            emit_load(1)
            eb01 = load_ebn(0, 1)
            emit_kproj(0, gs=(0,))
            emit_qproj(0, gs=(0,))
            st00 = pair_front(0, 0, ebn_pre=eb00, extras=[
                lambda: emit_gate(0, gs=(0,)),
                lambda: emit_gate(0, gs=(1,)),
                lambda: emit_vproj(0, range(0, 2)),
                lambda: emit_vproj(0, range(2, 4)),
            ])
            st01 = pair_front(0, 1, ebn_pre=eb01, extras=[
                lambda: emit_vproj(0, range(4, 6)),
                lambda: emit_vproj(0, range(6, 8)),
                lambda: emit_kproj(0, gs=(1,)),
                lambda: emit_qproj(0, gs=(1,)),
            ])
            emit_pv(st00)
            st02 = pair_front(0, 2, extras=[
                lambda: emit_kproj(1, gs=(0,)),
                lambda: emit_kproj(1, gs=(1,)),
                lambda: emit_qproj(1, gs=(0,)),
                lambda: emit_qproj(1, gs=(1,)),
            ])
            emit_pv(st01)
            st03 = pair_front(0, 3, extras=[
                lambda: emit_vproj(1, range(0, 2)),
                lambda: emit_vproj(1, range(2, 4)),
                lambda: emit_vproj(1, range(4, 6)),
                lambda: emit_vproj(1, range(6, 8)),
            ])
            emit_pv(st02)
            st10 = pair_front(1, 0, extras=[
                lambda: emit_gate(1, gs=(0,)),
                lambda: emit_gate(1, gs=(1,)),
            ])
            emit_pv(st03)
            pending = st10
            for p in range(1, 4):
                st = pair_front(1, p)
                emit_pv(pending)
                pending = st
            emit_pv(pending)
    _split_waits(nc)
    return nc


_CACHE = {}


def _get_runner():
    if "run" in _CACHE:
        return _CACHE["run"]
    import jax
    from jax.sharding import Mesh, PartitionSpec
    from jax.experimental.shard_map import shard_map
    from concourse import bass2jax

    nc = _build_nc()
    bass2jax.install_neuronx_cc_hook()

    in_names, out_names, out_avals, zero_outs = [], [], [], []
    partition_name = nc.partition_id_tensor.name if nc.partition_id_tensor else None
    for alloc in nc.m.functions[0].allocations:
        if not isinstance(alloc, mybir.MemoryLocationSet):
            continue
        name = alloc.memorylocations[0].name
        if alloc.kind == "ExternalInput":
            if name != partition_name:
                in_names.append(name)
        elif alloc.kind == "ExternalOutput":
            out_names.append(name)
            shape = tuple(alloc.tensor_shape)
            dtype = mybir.dt.np(alloc.dtype)
            out_avals.append(jax.core.ShapedArray(shape, dtype))
            zero_outs.append(np.zeros(shape, dtype))
    n_params = len(in_names)
    n_outs = len(out_avals)
    all_in = in_names + out_names + ([partition_name] if partition_name else [])

    def _body(*args):
        operands = list(args)
        if partition_name is not None:
            operands.append(bass2jax.partition_id_tensor())
        outs = bass2jax._bass_exec_p.bind(
            *operands, out_avals=tuple(out_avals), in_names=tuple(all_in),
            out_names=tuple(out_names), lowering_input_output_aliases=(),
            sim_require_finite=True, sim_require_nnan=True, nc=nc)
        return tuple(outs)

    devices = jax.devices()[:N_CORES]
    mesh = Mesh(np.asarray(devices), ("core",))
    in_specs = (PartitionSpec("core"),) * (n_params + n_outs)
    out_specs = (PartitionSpec("core"),) * n_outs
    donate = tuple(range(n_params, n_params + n_outs))
    sharded = jax.jit(
        shard_map(_body, mesh=mesh, in_specs=in_specs, out_specs=out_specs,
                  check_rep=False),
        donate_argnums=donate, keep_unused=True)

    def run(per_core_inputs):
        concat_in = [
            np.concatenate([per_core_inputs[c][nm] for c in range(N_CORES)], axis=0)
            for nm in in_names]
        concat_zeros = [
            np.zeros((N_CORES * z.shape[0], *z.shape[1:]), z.dtype)
            for z in zero_outs]
        out_arrs = sharded(*concat_in, *concat_zeros)
        return [
            {nm: np.asarray(out_arrs[i]).reshape(N_CORES, *out_avals[i].shape)[c]
             for i, nm in enumerate(out_names)}
            for c in range(N_CORES)]

    _CACHE["run"] = run
    _CACHE["nc"] = nc
    _CACHE["parts"] = (sharded, in_names, out_names, out_avals, zero_outs, mesh)
    return run


def _prep_inputs(q_data, m_data, bias, nonbatched_bias, query_w, key_w,
                 value_w, gating_w, gating_b, output_w, output_b):
    bf = ml_dtypes.bfloat16
    q_data = np.asarray(q_data, np.float32)
    m_data = np.asarray(m_data, np.float32)
    bias = np.asarray(bias, np.float32)
    nbb = np.asarray(nonbatched_bias, np.float32)
    wq = np.asarray(query_w, np.float32).reshape(D, H * DK)
    wk = np.asarray(key_w, np.float32).reshape(D, H * DK)
    wv = np.asarray(value_w, np.float32).reshape(D, H * DV)
    wg = np.asarray(gating_w, np.float32).reshape(D, H * DV)
    gb = np.asarray(gating_b, np.float32).reshape(H * DV)

    scale = DK ** -0.5
    wkq_c = np.concatenate([wk.reshape(2, 128, D), wq.reshape(2, 128, D)],
                           axis=2).astype(bf)
    wvg_c = np.concatenate([wv.reshape(2, 128, D), wg.reshape(2, 128, D)],
                           axis=2).astype(bf)
    gb_c = (0.5 * gb).reshape(2, 128, 1).astype(np.float32)

    eb_all = np.exp(bias[:, 0, :, :])          # [B, NQ, NK]
    en_all = np.exp(nbb)                       # [H, NQ, NK]

    per_core = []
    for c in range(N_CORES):
        i, j = c // GJ, c % GJ
        rows = slice(i * RQ, (i + 1) * RQ)
        bs = slice(j * BC, (j + 1) * BC)
        qt = (q_data[bs, rows, :] * scale).transpose(0, 2, 1)          # [BC, D, RQ]
        qt = np.ascontiguousarray(qt).reshape(BC, 2, 128, RQ)
        mt = m_data[bs].transpose(0, 2, 1).reshape(BC, 2, 128, NK)
        acts = np.concatenate([mt, qt], axis=3).astype(bf)   # [BC, 2, 128, NK+RQ]
        # ebn[b*4+p][part, h2*FKR + kc*RQ + q] = exp(bias[b,q,k] + nbb[h,q,k]),
        # k = kc*128 + part, h = 2p + h2 (pair-major for single 2MB DMAs)
        eb = eb_all[bs, rows, :].reshape(BC, 1, RQ, KC, 128)
        en = en_all[:, rows, :].reshape(1, H, RQ, KC, 128)
        ebn = (eb * en).transpose(0, 1, 4, 3, 2).reshape(BC * H, 128, FKR)
        ebn = ebn.reshape(BC * 4, 2, 128, FKR).transpose(0, 2, 1, 3)
        ebn = np.ascontiguousarray(ebn).reshape(BC * 4, 128, 2 * FKR).astype(bf)
        per_core.append({
            "acts": acts, "ebn": ebn,
            "wkq": wkq_c, "wvg": wvg_c, "gb": gb_c,
        })
    return per_core


def kernel(**inputs):
    per_core = _prep_inputs(**inputs)
    run = _get_runner()
    results = run(per_core)
    wo = np.asarray(inputs["output_w"], np.float32).reshape(H * DV, D)
    ob = np.asarray(inputs["output_b"], np.float32).reshape(D)
    # outw [BC, H//2, duo, 33, RQ]: rows 0-31 = 2*sig*wavg (unnormalized),
    # row 32 = 2*denominator
    wa = np.empty((B, NQ, H, DV), np.float32)
    for c in range(N_CORES):
        i, j = c // GJ, c % GJ
        o = results[c]["outw"].astype(np.float32)        # [BC, 4, 2, 33, RQ]
        w = o[:, :, :, 0:32, :]                          # [BC, 4, 2, 32, RQ]
        den = o[:, :, :, 32:33, :]                       # [BC, 4, 2, 1, RQ]
        # rows = 2*sig*wavg_unnorm, den row = 2*d -> rows/den = sig*wavg/d
        w = w / den                                      # [BC, 4, 2, 32, RQ]
        # heads h = p*2 + duo; -> [BC, RQ, H, DV]
        w = w.reshape(BC, H, DV, RQ).transpose(0, 3, 1, 2)
        wa[j * BC:(j + 1) * BC, i * RQ:(i + 1) * RQ] = w
    out = wa.reshape(B * NQ, H * DV) @ wo + ob
    return out.reshape(B, NQ, D).astype(np.float32)
